# revision 1
# baseline (speedup 1.0000x reference)
"""AttentionDGCNN Trainium2 kernel — 8 NeuronCores, TOKEN-sharded.

Core c owns tokens [128c, 128c+128) of ALL 8 samples. Consequences:
- The MHA (which attends across the batch axis at fixed token) is fully
  LOCAL: q,k,v for all 8 samples live on-core; no kv AllGather and no DMA
  staging in the attention loop.
- EdgeConv needs full-sample x as kNN columns / gather source, so each of
  layers 1-3 ends with an AllGather of the bf16 (O, 8, 128) activation
  shard (split in two 4-sample halves so the second half overlaps the
  next edgeconv's leading samples). ~600KB total collective traffic vs
  14MB for the batch-parallel kv-gather design.
- The head computes all samples redundantly on every core (tiny FCs)
  after one AllGather of per-core max/sum pooling partials.

Other key choices:
- Activations/weights bf16 for all big matmuls (PE full rate); PSUM
  accumulation f32. BatchNorm scales folded into weights host-side.
- kNN top-20 via packed score+index: the fp32 score's low 10 mantissa
  bits are replaced by the column index in ONE scalar_tensor_tensor op
  reading the score PSUM; per-128-column-segment max8 gives 64
  candidates; 3 max8/match_replace rounds give a rank-ordered top-24;
  the index is recovered with a u16 bitwise-and. Replaces the
  max/max_index/match_replace full-row rounds (8 passes -> ~2).
- The idx wrap for ap_gather round-trips DRAM per sample, so sample b's
  gather overlaps sample b+1's top-k.
- Neighbor-max over K=20: strided tensor_reduce (layers 1-3, fp32
  gather) / pairwise bf16 tensor_tensor max tree at 2x (layer 4, d=2
  two-plane gather).
- MHA: bf16 q*k / exp*v products (2x DVE), pairwise-tree reductions for
  the d-contraction and the m-sum, softmax denominator folded into one
  output scale. q-side work split in two sample halves.
- fp32r (RDT flag) is OFF: it crashed the exec unit on hardware
  (NRT_EXEC_UNIT_UNRECOVERABLE); bf16 paths made it unnecessary.
"""

import numpy as np

import concourse.bass as bass
import concourse.bacc as bacc
import concourse.mybir as mybir
from concourse.tile import TileContext
from concourse import bass_utils

F32 = mybir.dt.float32
F32R = mybir.dt.float32r
RDT = False                  # fp32r fast matmul mode (flaky on some HW paths)
BF16 = mybir.dt.bfloat16
U16 = mybir.dt.uint16
I16 = mybir.dt.int16
I32 = mybir.dt.int32
AX = mybir.AxisListType
ALU = mybir.AluOpType
ACTF = mybir.ActivationFunctionType

B, N, KNN, H = 8, 1024, 20, 4
TOK = 128                                          # tokens per core
NCORES = 8
SC = float(1.0 / np.sqrt(1.0 + 1e-5))
EC = [(3, 64), (64, 64), (64, 128), (128, 256)]    # (Cin, Cout) per edgeconv
ES = [64, 64, 128, 256]                            # MHA embed dims
RG = [list(range(NCORES))]
NEG = 0.2


class StopBuild(Exception):
    pass


# ----------------------------------------------------------------- host prep
def prep_weights(inp):
    """Fold BN scales, transpose for PE (lhsT layouts), build constants."""
    w = {}
    for i in range(1, 5):
        C, O = EC[i - 1]
        g = np.asarray(inp[f'g{i}']) * SC
        wi = np.asarray(inp[f'w{i}'])                 # (O, 2C)
        wn = wi[:, :C] * g[:, None]
        wz = (wi[:, C:] - wi[:, :C]) * g[:, None]
        w[f'ec{i}_wnT'] = np.ascontiguousarray(wn.T)  # (C, O)
        w[f'ec{i}_wzT'] = np.ascontiguousarray(wz.T)  # (C, O)
        nO = (O + 127) // 128
        b = np.zeros((nO * 128,), np.float32); b[:O] = np.asarray(inp[f'b{i}'])
        w[f'ec{i}_b'] = np.ascontiguousarray(b.reshape(nO, 128).T)  # (128, nO)
        w[f'ec{i}_b2'] = np.ascontiguousarray((-0.2 * b).reshape(nO, 128).T)

        E = ES[i - 1]
        d = E // H
        wiT = np.asarray(inp[f'a{i}_wi']).T.copy()    # (E, 3E)
        wiT[:, :E] *= 1.0 / np.sqrt(d)
        w[f'a{i}_wiT'] = np.ascontiguousarray(wiT)
        bi = np.asarray(inp[f'a{i}_bi']).copy()
        bi[:E] *= 1.0 / np.sqrt(d)
        w[f'a{i}_biR'] = np.ascontiguousarray(bi.reshape(1, 3 * E))
        w[f'a{i}_woT'] = np.ascontiguousarray(np.asarray(inp[f'a{i}_wo']).T)  # (E, E)
        ne = (E + 127) // 128
        bo = np.zeros((ne * 128,), np.float32); bo[:E] = np.asarray(inp[f'a{i}_bo'])
        w[f'a{i}_boc'] = np.ascontiguousarray(bo.reshape(ne, 128).T)

    g5 = np.asarray(inp['g5']) * SC
    w5T = np.ascontiguousarray((np.asarray(inp['w5']) * g5[:, None]).T)  # (512, 1024)
    w['w5T1'] = w5T[0:64]; w['w5T2'] = w5T[64:128]
    w['w5T3'] = w5T[128:256]; w['w5T4'] = w5T[256:512]
    w['b5c'] = np.ascontiguousarray(np.asarray(inp['b5']).reshape(8, 128).T)
    w['b5c2'] = np.ascontiguousarray(-0.2 * np.asarray(inp['b5']).reshape(8, 128).T)
    g6 = np.asarray(inp['g6']) * SC
    l1 = (np.asarray(inp['l1w']) * g6[:, None]).copy()     # (512, 2048)
    l1[:, 1024:] *= (1.0 / N)                              # fold mean 1/N
    w['l1wT'] = np.ascontiguousarray(l1.T)                 # (2048, 512)
    w['b6c'] = np.ascontiguousarray(np.asarray(inp['b6']).reshape(4, 128).T)
    w['b6c2'] = np.ascontiguousarray(-0.2 * np.asarray(inp['b6']).reshape(4, 128).T)
    g7 = np.asarray(inp['g7']) * SC
    w['l2wT'] = np.ascontiguousarray((np.asarray(inp['l2w']) * g7[:, None]).T)
    b2 = np.asarray(inp['l2b']) * g7 + np.asarray(inp['b7'])
    w['b2c'] = np.ascontiguousarray(b2.reshape(2, 128).T)
    w['b2c2'] = np.ascontiguousarray(-0.2 * b2.reshape(2, 128).T)
    w['l3wT'] = np.ascontiguousarray(np.asarray(inp['l3w']).T)  # (256, 40)
    b3 = np.zeros((128,), np.float32); b3[:40] = np.asarray(inp['l3b'])
    w['b3c'] = np.ascontiguousarray(b3.reshape(128, 1))
    w['id128'] = np.eye(128, dtype=np.float32)
    w['cones'] = np.ones((128, 1), np.float32)
    w['conesrow'] = np.ones((1, 128), np.float32)
    out = {}
    for k, v in w.items():
        dt = np.dtype('bfloat16') if k in BF16_W else np.float32
        out[k] = np.ascontiguousarray(np.asarray(v, np.float32).astype(dt))
    return out


HEAD_W = ('w5T1', 'w5T2', 'w5T3', 'w5T4', 'b5c', 'b5c2', 'l1wT', 'b6c', 'b6c2',
          'l2wT', 'b2c', 'b2c2', 'l3wT', 'b3c')
F32R_W = tuple(f'a{i}_biR' for i in range(1, 5)) + ('cones', 'conesrow')
BF16_W = tuple(f'ec{i}_{n}' for i in range(1, 5) for n in ('wnT', 'wzT')) + \
    tuple(f'a{i}_{n}' for i in range(1, 5) for n in ('wiT', 'woT')) + \
    ('w5T1', 'w5T2', 'w5T3', 'w5T4', 'id128')


def weight_specs():
    specs = {}
    for i in range(1, 5):
        C, O = EC[i - 1]
        E = ES[i - 1]
        nO = (O + 127) // 128
        ne = (E + 127) // 128
        specs.update({
            f'ec{i}_wnT': (C, O), f'ec{i}_wzT': (C, O), f'ec{i}_b': (128, nO),
            f'ec{i}_b2': (128, nO),
            f'a{i}_wiT': (E, 3 * E), f'a{i}_biR': (1, 3 * E),
            f'a{i}_woT': (E, E), f'a{i}_boc': (128, ne),
        })
    specs.update({
        'w5T1': (64, 1024), 'w5T2': (64, 1024), 'w5T3': (128, 1024),
        'w5T4': (256, 1024), 'b5c': (128, 8), 'b5c2': (128, 8),
        'l1wT': (2048, 512), 'b6c': (128, 4), 'b6c2': (128, 4),
        'l2wT': (512, 256), 'b2c': (128, 2), 'b2c2': (128, 2),
        'l3wT': (256, 40), 'b3c': (128, 1),
        'id128': (128, 128),
        'cones': (128, 1), 'conesrow': (1, 128),
    })
    return specs


# --------------------------------------------------------------- build kernel
class G:
    pass


def load_one_weight(g, pool, name, shape):
    nc = g.nc
    dram = g.win[name]
    dt = BF16 if name in BF16_W else (F32R if (RDT and name in F32R_W) else F32)
    if len(shape) == 2 and shape[0] > 128:
        kc = shape[0] // 128
        t = pool.tile([128, kc, shape[1]], dt, name=f'w_{name}')
        nc.sync.dma_start(out=t[:], in_=dram.ap().rearrange('(k p) n -> p k n', p=128))
        g.wkc[name] = kc
    else:
        t = pool.tile(list(shape), dt, name=f'w_{name}')
        nc.sync.dma_start(out=t[:], in_=dram.ap())
        g.wkc[name] = 0
    g.w[name] = t


def load_weights_sbuf(g):
    g.w = {}
    g.wkc = {}          # number of 128-row chunks for >128-row 2D weights
    for name, shape in weight_specs().items():
        if name in HEAD_W:
            continue
        load_one_weight(g, g.wpool, name, shape)


def wsl(g, name, r0, p, c0=None, cn=None):
    """Slice rows [r0:r0+p] (and cols [c0:c0+cn]) of a stored weight."""
    t = g.w[name]
    if g.wkc[name]:
        assert r0 % 128 + p <= 128
        ap = t[r0 % 128:r0 % 128 + p, r0 // 128, :]
    else:
        ap = t[r0:r0 + p, :]
    if c0 is not None:
        ap = ap[:, c0:c0 + cn]
    return ap


def dbg_emit(g, name, ap):
    if g.debug != name:
        return
    g.dbg_out = g.nc.dram_tensor('dbg', list(ap.shape), ap.dtype, kind='ExternalOutput')
    g.nc.sync.dma_start(out=g.dbg_out.ap(), in_=ap)
    raise StopBuild()


def stt2(g, out, in0, in1, op):
    """out = in0 <op> in1 on DVE (TensorTensor: 2x for packed 16-bit)."""
    g.nc.vector.tensor_tensor(out=out, in0=in0, in1=in1, op=op)


def tree_reduce(g, slicer, n, op):
    """In-place pairwise reduction along one axis; result lands at index 0.

    slicer(lo, hi) must return the AP slice [lo:hi) of the tree axis.
    """
    while n > 1:
        h = n // 2
        stt2(g, slicer(0, h), slicer(0, h), slicer(h, 2 * h), op)
        if n % 2:
            stt2(g, slicer(0, 1), slicer(0, 1), slicer(n - 1, n), op)
        n = h


def mm_acc(g, ps_ap, pairs, nmax=512, rdt=RDT):
    """Accumulate sum_i lhsT_i.T @ rhs_i into psum AP, splitting free dim.

    rdt=True runs the PE in float32r (fast fp32) mode.
    """
    nc = g.nc
    Nfree = pairs[0][1].shape[-1]
    for c0 in range(0, Nfree, nmax):
        cw = min(nmax, Nfree - c0)
        for j, (lt, rh) in enumerate(pairs):
            if rdt and lt.dtype == F32 and rh.dtype == F32:
                lt = lt.bitcast(F32R)
                rh = rh.bitcast(F32R)
            nc.tensor.matmul(
                out=ps_ap[:, c0:c0 + cw], lhsT=lt, rhs=rh[:, c0:c0 + cw],
                start=(j == 0), stop=(j == len(pairs) - 1))


def edgeconv(g, li, out_slices):
    """Token-sharded EdgeConv.

    Reads g.XF (C, 8, 1024) full activations and g.XL (list of local
    feature-major slices, total C rows of (rows, 8, 128)). Writes
    lrelu(max_k Wn x_nbr + Wz x_loc + b) into out_slices ((rows, 8, 128)).
    """
    nc, tc = g.nc, g.tc
    C, O = EC[li - 1]
    nO = (O + 127) // 128
    XF = g.XF                                          # (C, 8, 1024) AP

    with tc.tile_pool(name=f'ec{li}', bufs=1) as pool, \
         tc.tile_pool(name=f'ec{li}_ps', bufs=1, space='PSUM') as psp:

        # ---- 2*x_loc (lhsT for s)
        lhs2x = pool.tile([C, 8, TOK], BF16, name=f'l2x{li}')
        r = 0
        for xs in g.XL:
            p = xs.shape[0]
            nc.scalar.activation(out=lhs2x[r:r + p, :, :], in_=xs,
                                 func=ACTF.Copy, scale=2.0)
            r += p

        # ---- z = Wz @ x_loc (local tokens only)
        z_sb = pool.tile([128, nO, 8, TOK], F32, name=f'z_{li}')
        for oc in range(nO):
            ow = min(128, O - oc * 128)
            ps = psp.tile([128, 8 * TOK], F32, name=f'zps{li}', tag='sps', bufs=2)
            mm_acc(g, ps[0:ow, :],
                   [(wsl(g, f'ec{li}_wzT', 0, C, oc * 128, ow),
                     lhs2x.rearrange('c b n -> c (b n)'))])
            # lhs2x holds 2*x_loc; halve via scale on the copy out
            nc.scalar.activation(
                out=z_sb[0:ow, oc, :, :].rearrange('p b n -> p (b n)'),
                in_=ps[0:ow, :], func=ACTF.Copy, scale=0.5)

        # ---- per-sample pipeline: s -> packed top-k -> idx wrap -> y ->
        #      gather -> neighbor-max (wrap is per-sample so the gather of
        #      sample b overlaps the top-k of sample b+1)
        I24 = pool.tile([128, 8, 24], U16, name=f'i24_{li}')
        IDX = pool.tile([128, 8, 8 * KNN], I16, name=f'idx_{li}')
        m_sb = pool.tile([128, nO, 8, TOK], F32, name=f'm_{li}')
        for b in range(8):
            x2 = pool.tile([C, N], BF16, name=f'x2_{li}', tag='x2', bufs=2)
            nc.scalar.activation(out=x2[:], in_=XF[:, b, :], func=ACTF.Square)
            s_ps = psp.tile([128, N], F32, name=f's{li}', tag='sps', bufs=2)
            mm_acc(g, s_ps[:], [(lhs2x[:, b, :], XF[:, b, :]),
                                (g.negC16[0:C, :], x2[:])])
            if g.debug == f's{li}' and b == 0:
                dbg_emit(g, f's{li}', s_ps[:])
            SP = pool.tile([128, N], F32, name=f'sp_{li}', tag='sp', bufs=2)
            nc.vector.scalar_tensor_tensor(
                out=SP[:].bitcast(I32), in0=s_ps[:].bitcast(I32),
                scalar=g.maskhi[:], in1=g.iota[:],
                op0=ALU.bitwise_and, op1=ALU.bitwise_or)
            cand = pool.tile([128, 64], F32, name=f'cd_{li}', tag='cand', bufs=2)
            for seg in range(8):
                nc.vector.max(out=cand[:, seg * 8:seg * 8 + 8],
                              in_=SP[:, seg * 128:(seg + 1) * 128])
            P24 = pool.tile([128, 24], F32, name=f'p24_{li}', tag='p24', bufs=2)
            nc.vector.max(out=P24[:, 0:8], in_=cand[:])
            nc.vector.match_replace(out=cand[:], in_to_replace=P24[:, 0:8],
                                    in_values=cand[:], imm_value=-1e30)
            nc.vector.max(out=P24[:, 8:16], in_=cand[:])
            nc.vector.match_replace(out=cand[:], in_to_replace=P24[:, 8:16],
                                    in_values=cand[:], imm_value=-1e30)
            nc.vector.max(out=P24[:, 16:24], in_=cand[:])
            nc.vector.tensor_scalar(
                out=I24[:, b, :],
                in0=P24[:].bitcast(U16).rearrange(
                    'p (k two) -> p k two', two=2)[:, :, 0],
                scalar1=1023, scalar2=None, op0=ALU.bitwise_and)

            # wrap: IDX[16G+q, b, g*20+k] = idx[g*16+q, b, k]
            idx_dram = g.dram.tile([8, 16, 24], U16, name=f'idxd{li}_{b}')
            nc.sync.dma_start(
                out=idx_dram[:].rearrange('g q k -> (g q) k'),
                in_=I24[:, b, :])
            src_w = idx_dram[:, :, 0:KNN].rearrange(
                'g q k -> q g k').bitcast(I16)
            for Gi in range(8):
                dst = IDX[16 * Gi:16 * Gi + 16, b, :].rearrange(
                    'q (gg k) -> q gg k', gg=8, k=KNN)
                nc.sync.dma_start(out=dst, in_=src_w)

            # y for this sample, gather, neighbor-max
            if li < 4:
                y_sb = pool.tile([128, N], F32, name=f'y_{li}', tag='ysb',
                                 bufs=2)
                ps = psp.tile([128, N], F32, name=f'yps{li}', tag='yps', bufs=2)
                mm_acc(g, ps[0:O, :],
                       [(wsl(g, f'ec{li}_wnT', 0, C, 0, O), XF[:, b, :])])
                nc.scalar.activation(out=y_sb[0:O, :], in_=ps[0:O, :],
                                     func=ACTF.Copy)
                gth = pool.tile([128, TOK * KNN], F32, name=f'g_{li}',
                                tag='gth', bufs=2)
                nc.gpsimd.ap_gather(
                    out_ap=gth[0:O, :].rearrange('p (f o) -> p f o', o=1),
                    in_ap=y_sb[0:O, :].rearrange('p (f o) -> p f o', o=1),
                    idxs_ap=IDX[0:O, b, :],
                    channels=O, num_elems=N, d=1, num_idxs=TOK * KNN)
                nc.vector.tensor_reduce(
                    out=m_sb[0:O, 0, b, :].rearrange('p (gg q) -> p gg q',
                                                     gg=8),
                    in_=gth[0:O, :].rearrange('p (gg k q) -> p gg q k',
                                              gg=8, k=KNN, q=16),
                    axis=AX.X, op=ALU.max)
            else:
                # layer 4: two oc planes interleaved, bf16, one d=2 gather
                y_sb = pool.tile([128, N, 2], BF16, name=f'y_{li}',
                                 tag='ysb', bufs=2)
                for oc in range(2):
                    ps = psp.tile([128, N], F32, name=f'yps{li}', tag='yps',
                                  bufs=2)
                    mm_acc(g, ps[:],
                           [(wsl(g, f'ec{li}_wnT', 0, C, oc * 128, 128),
                             XF[:, b, :])])
                    nc.scalar.activation(out=y_sb[:, :, oc], in_=ps[:],
                                         func=ACTF.Copy)
                gth = pool.tile([128, TOK * KNN, 2], BF16, name=f'g_{li}',
                                tag='gth', bufs=2)
                nc.gpsimd.ap_gather(
                    out_ap=gth[:], in_ap=y_sb[:],
                    idxs_ap=IDX[:, b, :],
                    channels=128, num_elems=N, d=2, num_idxs=TOK * KNN)
                gv = gth[:].rearrange('p (gg k q) two -> p gg k q two',
                                      gg=8, k=KNN, q=16)
                tree_reduce(g, lambda lo, hi: gv[:, :, lo:hi, :, :],
                            KNN, ALU.max)
                stt2(g, m_sb[:, :, b, :].rearrange(
                         'p two (gg q) -> p gg q two', gg=8),
                     gv[:, :, 0, :, :], gv[:, :, 0, :, :], ALU.max)
        dbg_emit(g, f'i24_{li}', I24[:])
        dbg_emit(g, f'idx_{li}', IDX[:])
        dbg_emit(g, f'm_{li}', m_sb[0:min(O, 128), :, :, :])

        # ---- epilogue: lrelu(m + z + b) -> out slices, per sample-half so
        #      the next stage's qkv can start while late gathers still run
        for hb in range(2):
            bsl = slice(hb * 4, hb * 4 + 4)
            for oc in range(nO):
                ow = min(128, O - oc * 128)
                t_sb = pool.tile([128, 4, TOK], F32, name=f't_{li}',
                                 tag='tsb', bufs=2)
                stt2(g, t_sb[0:ow, :, :], m_sb[0:ow, oc, bsl, :],
                     z_sb[0:ow, oc, bsl, :], ALU.add)
                neg = pool.tile([128, 4, TOK], BF16, name=f'neg_{li}',
                                tag='negt', bufs=2)
                r = 0
                for os_ in out_slices:
                    p = os_.shape[0]
                    lo, hi = max(r, oc * 128), min(r + p, oc * 128 + ow)
                    if lo < hi:
                        sl = slice(lo - oc * 128, hi - oc * 128)
                        nc.scalar.activation(
                            out=os_[lo - r:hi - r, bsl, :], in_=t_sb[sl, :, :],
                            func=ACTF.Relu,
                            bias=g.w[f'ec{li}_b'][sl, oc:oc + 1])
                        nc.scalar.activation(
                            out=neg[sl, :, :], in_=t_sb[sl, :, :],
                            func=ACTF.Relu, scale=-0.2,
                            bias=g.w[f'ec{li}_b2'][sl, oc:oc + 1])
                        stt2(g, os_[lo - r:hi - r, bsl, :],
                             os_[lo - r:hi - r, bsl, :],
                             neg[sl, :, :], ALU.subtract)
                    r += p


def mha(g, li, x_slices, on_half=None):
    """Local cross-batch MHA + residual, in place on x_slices ((rows, 8, 128)).

    Token-major: qkv (128tok, 8sample, 3E). Attention mixes over the sample
    axis m (all resident). After qkv, the q-side work runs in two sample
    halves; on_half(h) fires when a half's residual is in place so its
    AllGather can overlap the other half's compute.
    """
    nc, tc = g.nc, g.tc
    E = ES[li - 1]
    d = E // H
    ne = (E + 127) // 128

    with tc.tile_pool(name=f'mha{li}', bufs=1) as pool, \
         tc.tile_pool(name=f'mha{li}_ps', bufs=1, space='PSUM') as psp:

        # ---- qkv[p, b, o] = sum_e x[e, b, p] wiT[e, o] + bi[o]  (bf16)
        qkv = pool.tile([128, 8, 3 * E], BF16, name=f'qkv_{li}')
        for b in range(8):
            ps = psp.tile([128, 3 * E], F32, name=f'qk{li}', tag='qkvps', bufs=2)
            pairs = []
            r = 0
            for xs in x_slices:
                p = xs.shape[0]
                pairs.append((xs[:, b, :], wsl(g, f'a{li}_wiT', r, p)))
                r += p
            pairs.append((g.onesrow[:], g.w[f'a{li}_biR'][:]))
            mm_acc(g, ps[:], pairs)
            nc.scalar.activation(out=qkv[:, b, :], in_=ps[:], func=ACTF.Copy)
        dbg_emit(g, f'qkv_{li}', qkv[:])

        q = qkv[:, :, 0:E]
        k = qkv[:, :, E:2 * E]
        v = qkv[:, :, 2 * E:3 * E]
        o_fm = pool.tile([128, ne, 8, TOK], BF16, name=f'of_{li}')

        for half in range(2):
            b0 = half * 4
            bsl = slice(b0, b0 + 4)

            # scores: prod = q*k (bf16 2x), then pairwise d-tree sum
            prod = pool.tile([128, 4, 8, E], BF16, name=f'pr_{li}',
                             tag='prod', bufs=2)
            stt2(g, prod[:],
                 q[:, bsl, :].rearrange('p b (o e) -> p b o e', o=1)
                 .to_broadcast([128, 4, 8, E]),
                 k.rearrange('p m (o e) -> p o m e', o=1).to_broadcast(
                     [128, 4, 8, E]),
                 ALU.mult)
            pv = prod[:].rearrange('p b m (h dd) -> p b m h dd', h=H)
            tree_reduce(g, lambda lo, hi: pv[:, :, :, :, lo:hi], d, ALU.add)
            st = pv[:, :, :, :, 0]                   # (128, 4, 8, H) strided

            # softmax over m (denominator folded into the output scale)
            ex = pool.tile([128, 4, 8, H], BF16, name=f'ex_{li}',
                           tag='ex', bufs=2)
            nc.scalar.activation(out=ex[:], in_=st, func=ACTF.Exp)
            sm = pool.tile([128, 4, H], F32, name=f'sm_{li}', tag='sm', bufs=2)
            nc.vector.tensor_reduce(
                out=sm[:], in_=ex[:].rearrange('p b m h -> p b h m'),
                axis=AX.X, op=ALU.add)
            rec = pool.tile([128, 4, H], F32, name=f'rc_{li}', tag='rc', bufs=2)
            nc.vector.reciprocal(out=rec[:], in_=sm[:])

            # o[p, b, e] = (sum_m ex[b,m,h(e)] v[m,e]) * rec[b,h(e)]
            prod2 = pool.tile([128, 4, 8, E], BF16, name=f'p2_{li}',
                              tag='prod', bufs=2)
            stt2(g, prod2[:].rearrange('p b m (h dd) -> p b m h dd', h=H),
                 v.rearrange('p m (o h dd) -> p o m h dd', o=1, h=H)
                 .to_broadcast([128, 4, 8, H, d]),
                 ex[:].rearrange('p b m (h o) -> p b m h o', o=1).to_broadcast(
                     [128, 4, 8, H, d]),
                 ALU.mult)
            tree_reduce(g, lambda lo, hi: prod2[:, :, lo:hi, :], 8, ALU.add)
            o_t = pool.tile([128, 4, E], BF16, name=f'o_{li}', tag='ot', bufs=2)
            stt2(g, o_t[:].rearrange('p b (h dd) -> p b h dd', h=H),
                 prod2[:, :, 0, :].rearrange('p b (h dd) -> p b h dd', h=H),
                 rec[:].rearrange('p b (h o) -> p b h o', o=1).to_broadcast(
                     [128, 4, H, d]),
                 ALU.mult)

            # transpose o to feature-major, project, bias, residual
            for ec in range(ne):
                p = min(128, E - ec * 128)
                po = psp.tile([128, 4 * TOK], BF16, name=f'po{li}',
                              tag='pops', bufs=1)
                for j in range(4):
                    nc.tensor.transpose(out=po[0:p, j * TOK:(j + 1) * TOK],
                                        in_=o_t[:, j, ec * 128:ec * 128 + p],
                                        identity=g.w['id128'][:])
                nc.scalar.activation(
                    out=o_fm[0:p, ec, bsl, :].rearrange('p b n -> p (b n)'),
                    in_=po[0:p, :], func=ACTF.Copy)
            for oc in range(ne):
                p = min(128, E - oc * 128)
                ps = psp.tile([128, 4 * TOK], F32, name=f'pj{li}',
                              tag='pjps', bufs=2)
                pairs = []
                for ec in range(ne):
                    pk = min(128, E - ec * 128)
                    pairs.append(
                        (wsl(g, f'a{li}_woT', ec * 128, pk, oc * 128, p),
                         o_fm[0:pk, ec, bsl, :].rearrange('p b n -> p (b n)')))
                mm_acc(g, ps[0:p, :], pairs)
                tmp = pool.tile([128, 4, TOK], BF16, name=f'pt_{li}',
                                tag='ptmp', bufs=2)
                nc.scalar.activation(
                    out=tmp[0:p, :, :].rearrange('p b n -> p (b n)'),
                    in_=ps[0:p, :], func=ACTF.Identity,
                    bias=g.w[f'a{li}_boc'][0:p, oc:oc + 1])
                r = 0
                for xs in x_slices:
                    pp = xs.shape[0]
                    lo, hi = max(r, oc * 128), min(r + pp, oc * 128 + p)
                    if lo < hi:
                        stt2(g, xs[lo - r:hi - r, bsl, :],
                             tmp[lo - oc * 128:hi - oc * 128, :, :],
                             xs[lo - r:hi - r, bsl, :], ALU.add)
                    r += pp
            if on_half is not None:
                on_half(half)


def ag_half(g, li, out_slices, XFc, half):
    """AllGather one 4-sample half of the bf16 (O, 8, 128) local shard into
    XFc (O, 8, 1024). Fired from mha's on_half so the first half's
    collective overlaps the second half's attention compute, and the second
    overlaps the next edgeconv's leading samples."""
    nc = g.nc
    xs = out_slices[0]
    rows = xs.shape[0]
    b0 = half * 4
    xc = g.dram.tile([rows, 4, TOK], BF16, name=f'xc{li}_{half}')
    nc.sync.dma_start(out=xc[:], in_=xs[:, b0:b0 + 4, :])
    x_all = g.dram.tile([NCORES, rows, 4, TOK], BF16,
                        name=f'xall{li}_{half}', addr_space='Shared')
    nc.gpsimd.collective_compute(
        'AllGather', ALU.bypass, replica_groups=RG,
        ins=[xc[:].opt()], outs=[x_all[:].opt()])
    nc.sync.dma_start(
        out=XFc[0:rows, b0:b0 + 4, :].rearrange(
            'o b (c n) -> o b c n', c=NCORES),
        in_=x_all[:].rearrange('c o b n -> o b c n'))


def head(g):
    nc, tc = g.nc, g.tc
    specs = weight_specs()
    with tc.tile_pool(name='head', bufs=1) as pool, \
         tc.tile_pool(name='head_ps', bufs=1, space='PSUM') as psp:
        for name in HEAD_W:
            load_one_weight(g, pool, name, specs[name])
        cat = g.cat
        h_sb = pool.tile([128, 8, 8, TOK], F32, name='h5')
        hneg = pool.tile([128, 8, TOK], F32, name='h5n', tag='h5n', bufs=2)
        for oc in range(8):
            ps = psp.tile([128, 8 * TOK], F32, name='h5_ps', tag='h5ps', bufs=2)
            cs = slice(oc * 128, oc * 128 + 128)
            pairs = [(g.w['w5T1'][:, cs], cat[0:64, 0, :, :].rearrange(
                        'p b n -> p (b n)')),
                     (g.w['w5T2'][:, cs], cat[0:64, 1, :, :].rearrange(
                        'p b n -> p (b n)')),
                     (g.w['w5T3'][:, cs], cat[0:128, 2, :, :].rearrange(
                        'p b n -> p (b n)')),
                     (g.w['w5T4'][:, 0, cs], cat[0:128, 3, :, :].rearrange(
                        'p b n -> p (b n)')),
                     (g.w['w5T4'][:, 1, cs], cat[0:128, 4, :, :].rearrange(
                        'p b n -> p (b n)'))]
            mm_acc(g, ps[:], pairs)
            nc.scalar.activation(
                out=h_sb[:, oc, :, :].rearrange('p b n -> p (b n)'), in_=ps[:],
                func=ACTF.Relu, bias=g.w['b5c'][:, oc:oc + 1])
            nc.scalar.activation(
                out=hneg[:].rearrange('p b n -> p (b n)'), in_=ps[:],
                func=ACTF.Relu, scale=-0.2, bias=g.w['b5c2'][:, oc:oc + 1])
            stt2(g, h_sb[:, oc, :, :], h_sb[:, oc, :, :], hneg[:], ALU.subtract)
        # per-sample local max / sum partials over the 128 local tokens
        hp = pool.tile([128, 8, 8, 2], F32, name='hpart')
        for oc in range(8):
            nc.vector.tensor_reduce(out=hp[:, oc, :, 0],
                                    in_=h_sb[:, oc, :, :], axis=AX.X, op=ALU.max)
            nc.vector.tensor_reduce(out=hp[:, oc, :, 1],
                                    in_=h_sb[:, oc, :, :], axis=AX.X, op=ALU.add)
        dbg_emit(g, 'hpart', hp[:])
        hc = g.dram.tile([128, 8, 8, 2], F32, name='hc')
        nc.sync.dma_start(out=hc[:], in_=hp[:])
        h_all = g.dram.tile([NCORES, 128, 8, 8, 2], F32, name='hall',
                            addr_space='Shared')
        nc.gpsimd.collective_compute(
            'AllGather', ALU.bypass, replica_groups=RG,
            ins=[hc[:].opt()], outs=[h_all[:].opt()])
        hl = pool.tile([128, NCORES, 8, 8, 2], F32, name='hload')
        nc.sync.dma_start(out=hl[:],
                          in_=h_all[:].rearrange('c p o b s -> p c o b s'))
        hmax = pool.tile([128, 8, 8], F32, name='hmax')
        hsum = pool.tile([128, 8, 8], F32, name='hsum')
        nc.vector.tensor_reduce(
            out=hmax[:], in_=hl[:, :, :, :, 0].rearrange('p c o b -> p o b c'),
            axis=AX.X, op=ALU.max)
        nc.vector.tensor_reduce(
            out=hsum[:], in_=hl[:, :, :, :, 1].rearrange('p c o b -> p o b c'),
            axis=AX.X, op=ALU.add)
        dbg_emit(g, 'hmax', hmax[:])
        dbg_emit(g, 'hsum', hsum[:])

        fc1 = pool.tile([128, 4, 8], F32, name='fc1')
        for oc in range(4):
            ps = psp.tile([128, 8], F32, name='fc_ps', tag='fcps', bufs=2)
            for kc in range(16):
                rhs = hmax[:, kc, :] if kc < 8 else hsum[:, kc - 8, :]
                nc.tensor.matmul(out=ps[:],
                                 lhsT=g.w['l1wT'][:, kc, oc * 128:oc * 128 + 128],
                                 rhs=rhs, start=(kc == 0), stop=(kc == 15))
            fneg = pool.tile([128, 8], F32, name='fc1n', tag='fcn', bufs=2)
            nc.scalar.activation(out=fc1[:, oc, :], in_=ps[:], func=ACTF.Relu,
                                 bias=g.w['b6c'][:, oc:oc + 1])
            nc.scalar.activation(out=fneg[:], in_=ps[:], func=ACTF.Relu, scale=-0.2,
                                 bias=g.w['b6c2'][:, oc:oc + 1])
            nc.vector.tensor_tensor(out=fc1[:, oc, :], in0=fc1[:, oc, :],
                                    in1=fneg[:], op=ALU.subtract)
        fc2 = pool.tile([128, 2, 8], F32, name='fc2')
        for oc in range(2):
            ps = psp.tile([128, 8], F32, name='fc_ps2', tag='fcps', bufs=2)
            for kc in range(4):
                nc.tensor.matmul(out=ps[:],
                                 lhsT=g.w['l2wT'][:, kc, oc * 128:oc * 128 + 128],
                                 rhs=fc1[:, kc, :], start=(kc == 0), stop=(kc == 3))
            fneg = pool.tile([128, 8], F32, name='fc2n', tag='fcn', bufs=2)
            nc.scalar.activation(out=fc2[:, oc, :], in_=ps[:], func=ACTF.Relu,
                                 bias=g.w['b2c'][:, oc:oc + 1])
            nc.scalar.activation(out=fneg[:], in_=ps[:], func=ACTF.Relu, scale=-0.2,
                                 bias=g.w['b2c2'][:, oc:oc + 1])
            nc.vector.tensor_tensor(out=fc2[:, oc, :], in0=fc2[:, oc, :],
                                    in1=fneg[:], op=ALU.subtract)
        ps = psp.tile([128, 8], F32, name='fc_ps3', tag='fcps', bufs=2)
        for kc in range(2):
            nc.tensor.matmul(out=ps[0:40, :], lhsT=g.w['l3wT'][:, kc, 0:40],
                             rhs=fc2[:, kc, :], start=(kc == 0), stop=(kc == 1))
        outs = pool.tile([128, 8], F32, name='outs')
        nc.scalar.activation(out=outs[0:40, :], in_=ps[0:40, :], func=ACTF.Identity,
                             bias=g.w['b3c'][0:40, :])
        nc.sync.dma_start(out=g.out.ap(), in_=outs[0:40, :])


def build(debug=None):
    g = G()
    g.debug = debug
    nc = bacc.Bacc('TRN2', target_bir_lowering=False, debug=False,
                   num_devices=NCORES)
    g.nc = nc
    g.win = {}
    g.win['xfull'] = nc.dram_tensor('xfull', [3, 8, N], BF16, kind='ExternalInput')
    g.win['xloc'] = nc.dram_tensor('xloc', [3, 8, TOK], BF16, kind='ExternalInput')
    for name, shape in weight_specs().items():
        dt = BF16 if name in BF16_W else (F32R if (RDT and name in F32R_W) else F32)
        g.win[name] = nc.dram_tensor(name, list(shape), dt, kind='ExternalInput')
    if debug is None:
        g.out = nc.dram_tensor('out', [40, 8], F32, kind='ExternalOutput')

    with TileContext(nc) as tc:
        g.tc = tc
        with tc.tile_pool(name='wpool', bufs=1) as wpool, \
             tc.tile_pool(name='gpool', bufs=1) as gpool, \
             tc.tile_pool(name='dram', bufs=1, space='DRAM') as dram:
            g.wpool, g.dram = wpool, dram
            try:
                load_weights_sbuf(g)
                g.ones = g.w['cones']
                g.negC16 = gpool.tile([128, 128], BF16, name='negC16')
                nc.vector.memset(g.negC16[:], -1.0)
                g.onesrow = g.w['conesrow']
                g.iota = gpool.tile([128, N], I32, name='iota')
                nc.gpsimd.iota(g.iota[:], pattern=[[1, N]], channel_multiplier=0)
                g.maskhi = gpool.tile([128, 1], I32, name='maskhi')
                nc.vector.memset(g.maskhi[:], -1024)       # 0xFFFFFC00
                g.masklo = gpool.tile([128, 1], I32, name='masklo')
                nc.vector.memset(g.masklo[:], 1023)
                g.cat = gpool.tile([128, 5, 8, TOK], BF16, name='cat')
                if debug is not None:
                    nc.vector.memset(g.cat[:], 0.0)

                x1 = [g.cat[0:64, 0, :, :]]
                x2 = [g.cat[0:64, 1, :, :]]
                x3 = [g.cat[0:128, 2, :, :]]
                x4 = [g.cat[0:128, 3, :, :], g.cat[0:128, 4, :, :]]

                xfctx = tc.tile_pool(name='xfpool', bufs=1)
                xfpool = xfctx.__enter__()
                XF2c = xfpool.tile([64, 8, N], BF16, name='XF2c')
                XF3c = xfpool.tile([64, 8, N], BF16, name='XF3c')
                XF4c = xfpool.tile([128, 8, N], BF16, name='XF4c')

                with tc.tile_pool(name='l1', bufs=1) as p1:
                    XF1 = p1.tile([3, 8, N], BF16, name='XF1')
                    nc.sync.dma_start(out=XF1[:], in_=g.win['xfull'].ap())
                    XL1 = p1.tile([3, 8, TOK], BF16, name='XL1')
                    nc.sync.dma_start(out=XL1[:], in_=g.win['xloc'].ap())
                    g.XF, g.XL = XF1[:], [XL1[:]]
                    edgeconv(g, 1, x1)
                    dbg_emit(g, 'x1', g.cat[:])
                    mha(g, 1, x1,
                        on_half=lambda h: ag_half(g, 1, x1, XF2c, h))
                    dbg_emit(g, 'xa1', g.cat[:])

                g.XF, g.XL = XF2c[0:64, :, :], x1
                edgeconv(g, 2, x2)
                dbg_emit(g, 'x2', g.cat[:])
                mha(g, 2, x2, on_half=lambda h: ag_half(g, 2, x2, XF3c, h))
                dbg_emit(g, 'xa2', g.cat[:])

                g.XF, g.XL = XF3c[0:64, :, :], x2
                edgeconv(g, 3, x3)
                dbg_emit(g, 'x3', g.cat[:])
                mha(g, 3, x3, on_half=lambda h: ag_half(g, 3, x3, XF4c, h))
                dbg_emit(g, 'xa3', g.cat[:])

                g.XF, g.XL = XF4c[0:128, :, :], x3
                edgeconv(g, 4, x4)
                dbg_emit(g, 'x4', g.cat[:])
                mha(g, 4, x4)
                dbg_emit(g, 'cat', g.cat[:])
                xfctx.__exit__(None, None, None)
                head(g)
            except StopBuild:
                pass
    nc.compile()
    return g


# ------------------------------------------------------------------ host run
def make_in_maps(inputs):
    w = prep_weights(inputs)
    x = np.asarray(inputs['x'], np.float32)           # (8, 1024, 3)
    xf = np.ascontiguousarray(x.transpose(2, 0, 1)).astype(
        np.dtype('bfloat16'))                         # (3, 8, 1024)
    in_maps = []
    for c in range(NCORES):
        m = {'xfull': xf,
             'xloc': np.ascontiguousarray(xf[:, :, c * TOK:(c + 1) * TOK])}
        m.update(w)
        in_maps.append(m)
    return in_maps


def kernel(**inputs):
    g = build()
    in_maps = make_in_maps(inputs)
    res = bass_utils.run_bass_kernel_spmd(g.nc, in_maps, core_ids=list(range(NCORES)))
    return np.ascontiguousarray(
        np.asarray(res.results[0]['out']).reshape(40, 8).T).astype(np.float32)



# revision 6
# speedup vs baseline: 1.1117x; 1.1117x over previous
"""AttentionDGCNN Trainium2 kernel — 8 NeuronCores, TOKEN-sharded.

Core c owns tokens [128c, 128c+128) of ALL 8 samples. Consequences:
- The MHA (which attends across the batch axis at fixed token) is fully
  LOCAL: q,k,v for all 8 samples live on-core; no kv AllGather and no DMA
  staging in the attention loop.
- EdgeConv needs full-sample x as kNN columns / gather source, so each of
  layers 1-3 ends with an AllGather of the bf16 (O, 8, 128) activation
  shard (split in two 4-sample halves so the second half overlaps the
  next edgeconv's leading samples). ~600KB total collective traffic vs
  14MB for the batch-parallel kv-gather design.
- The head computes all samples redundantly on every core (tiny FCs)
  after one AllGather of per-core max/sum pooling partials.

Other key choices:
- Activations/weights bf16 for all big matmuls (PE full rate); PSUM
  accumulation f32. BatchNorm scales folded into weights host-side.
- kNN top-20 via packed score+index: the fp32 score's low 10 mantissa
  bits are replaced by the column index in ONE scalar_tensor_tensor op
  reading the score PSUM; per-128-column-segment max8 gives 64
  candidates; 3 max8/match_replace rounds give a rank-ordered top-24;
  the index is recovered with a u16 bitwise-and. Replaces the
  max/max_index/match_replace full-row rounds (8 passes -> ~2).
- The idx wrap for ap_gather round-trips DRAM per sample, so sample b's
  gather overlaps sample b+1's top-k.
- Neighbor-max over K=20: strided tensor_reduce (layers 1-3, fp32
  gather) / pairwise bf16 tensor_tensor max tree at 2x (layer 4, d=2
  two-plane gather).
- MHA: bf16 q*k / exp*v products (2x DVE), pairwise-tree reductions for
  the d-contraction and the m-sum, softmax denominator folded into one
  output scale. q-side work split in two sample halves.
- fp32r (RDT flag) is OFF: it crashed the exec unit on hardware
  (NRT_EXEC_UNIT_UNRECOVERABLE); bf16 paths made it unnecessary.
"""

import numpy as np

import concourse.bass as bass
import concourse.bacc as bacc
import concourse.mybir as mybir
from concourse.tile import TileContext
from concourse import bass_utils

F32 = mybir.dt.float32
F32R = mybir.dt.float32r
RDT = False                  # fp32r fast matmul mode (flaky on some HW paths)
BF16 = mybir.dt.bfloat16
U16 = mybir.dt.uint16
I16 = mybir.dt.int16
I32 = mybir.dt.int32
AX = mybir.AxisListType
ALU = mybir.AluOpType
ACTF = mybir.ActivationFunctionType

B, N, KNN, H = 8, 1024, 20, 4
TOK = 128                                          # tokens per core
NCORES = 8
SC = float(1.0 / np.sqrt(1.0 + 1e-5))
EC = [(3, 64), (64, 64), (64, 128), (128, 256)]    # (Cin, Cout) per edgeconv
ES = [64, 64, 128, 256]                            # MHA embed dims
RG = [list(range(NCORES))]
NEG = 0.2


class StopBuild(Exception):
    pass


# ----------------------------------------------------------------- host prep
def prep_weights(inp):
    """Fold BN scales, transpose for PE (lhsT layouts), build constants."""
    w = {}
    for i in range(1, 5):
        C, O = EC[i - 1]
        g = np.asarray(inp[f'g{i}']) * SC
        wi = np.asarray(inp[f'w{i}'])                 # (O, 2C)
        wn = wi[:, :C] * g[:, None]
        wz = (wi[:, C:] - wi[:, :C]) * g[:, None]
        w[f'ec{i}_wnT'] = np.ascontiguousarray(wn.T)  # (C, O)
        w[f'ec{i}_wzT'] = np.ascontiguousarray(wz.T)  # (C, O)
        nO = (O + 127) // 128
        b = np.zeros((nO * 128,), np.float32); b[:O] = np.asarray(inp[f'b{i}'])
        w[f'ec{i}_b'] = np.ascontiguousarray(b.reshape(nO, 128).T)  # (128, nO)
        w[f'ec{i}_b2'] = np.ascontiguousarray((-0.2 * b).reshape(nO, 128).T)

        E = ES[i - 1]
        d = E // H
        wiT = np.asarray(inp[f'a{i}_wi']).T.copy()    # (E, 3E)
        wiT[:, :E] *= 1.0 / np.sqrt(d)
        w[f'a{i}_wiT'] = np.ascontiguousarray(wiT)
        bi = np.asarray(inp[f'a{i}_bi']).copy()
        bi[:E] *= 1.0 / np.sqrt(d)
        w[f'a{i}_biR'] = np.ascontiguousarray(bi.reshape(1, 3 * E))
        w[f'a{i}_woT'] = np.ascontiguousarray(np.asarray(inp[f'a{i}_wo']).T)  # (E, E)
        ne = (E + 127) // 128
        bo = np.zeros((ne * 128,), np.float32); bo[:E] = np.asarray(inp[f'a{i}_bo'])
        w[f'a{i}_boc'] = np.ascontiguousarray(bo.reshape(ne, 128).T)

    g5 = np.asarray(inp['g5']) * SC
    w5T = np.ascontiguousarray((np.asarray(inp['w5']) * g5[:, None]).T)  # (512, 1024)
    w['w5T1'] = w5T[0:64]; w['w5T2'] = w5T[64:128]
    w['w5T3'] = w5T[128:256]; w['w5T4'] = w5T[256:512]
    w['b5c'] = np.ascontiguousarray(np.asarray(inp['b5']).reshape(8, 128).T)
    w['b5c2'] = np.ascontiguousarray(-0.2 * np.asarray(inp['b5']).reshape(8, 128).T)
    g6 = np.asarray(inp['g6']) * SC
    l1 = (np.asarray(inp['l1w']) * g6[:, None]).copy()     # (512, 2048)
    l1[:, 1024:] *= (1.0 / N)                              # fold mean 1/N
    w['l1wT'] = np.ascontiguousarray(l1.T)                 # (2048, 512)
    w['b6c'] = np.ascontiguousarray(np.asarray(inp['b6']).reshape(4, 128).T)
    w['b6c2'] = np.ascontiguousarray(-0.2 * np.asarray(inp['b6']).reshape(4, 128).T)
    g7 = np.asarray(inp['g7']) * SC
    w['l2wT'] = np.ascontiguousarray((np.asarray(inp['l2w']) * g7[:, None]).T)
    b2 = np.asarray(inp['l2b']) * g7 + np.asarray(inp['b7'])
    w['b2c'] = np.ascontiguousarray(b2.reshape(2, 128).T)
    w['b2c2'] = np.ascontiguousarray(-0.2 * b2.reshape(2, 128).T)
    w['l3wT'] = np.ascontiguousarray(np.asarray(inp['l3w']).T)  # (256, 40)
    b3 = np.zeros((128,), np.float32); b3[:40] = np.asarray(inp['l3b'])
    w['b3c'] = np.ascontiguousarray(b3.reshape(128, 1))
    w['id128'] = np.eye(128, dtype=np.float32)
    w['cones'] = np.ones((128, 1), np.float32)
    w['conesrow'] = np.ones((1, 128), np.float32)
    out = {}
    for k, v in w.items():
        dt = np.dtype('bfloat16') if k in BF16_W else np.float32
        out[k] = np.ascontiguousarray(np.asarray(v, np.float32).astype(dt))
    return out


HEAD_W = ('w5T1', 'w5T2', 'w5T3', 'w5T4', 'b5c', 'b5c2', 'l1wT', 'b6c', 'b6c2',
          'l2wT', 'b2c', 'b2c2', 'l3wT', 'b3c')
F32R_W = tuple(f'a{i}_biR' for i in range(1, 5)) + ('cones', 'conesrow')
BF16_W = tuple(f'ec{i}_{n}' for i in range(1, 5) for n in ('wnT', 'wzT')) + \
    tuple(f'a{i}_{n}' for i in range(1, 5) for n in ('wiT', 'woT')) + \
    ('w5T1', 'w5T2', 'w5T3', 'w5T4', 'id128')

BLOB_ALIGN = 128        # element alignment of each weight inside its blob


def blob_layout():
    """All weights pack into two 1-D DRAM blobs (bf16 / f32) so the runtime
    binds 4 input tensors per dispatch instead of 46 (binding costs ~18us
    per tensor per call through the PJRT path)."""
    lay = {}
    off = {'bf16': 0, 'f32': 0}
    for name, shape in weight_specs().items():
        kind = 'bf16' if name in BF16_W else 'f32'
        n = int(np.prod(shape))
        lay[name] = (kind, off[kind], shape)
        off[kind] += ((n + BLOB_ALIGN - 1) // BLOB_ALIGN) * BLOB_ALIGN
    return lay, off


def weight_specs():
    specs = {}
    for i in range(1, 5):
        C, O = EC[i - 1]
        E = ES[i - 1]
        nO = (O + 127) // 128
        ne = (E + 127) // 128
        specs.update({
            f'ec{i}_wnT': (C, O), f'ec{i}_wzT': (C, O), f'ec{i}_b': (128, nO),
            f'ec{i}_b2': (128, nO),
            f'a{i}_wiT': (E, 3 * E), f'a{i}_biR': (1, 3 * E),
            f'a{i}_woT': (E, E), f'a{i}_boc': (128, ne),
        })
    specs.update({
        'w5T1': (64, 1024), 'w5T2': (64, 1024), 'w5T3': (128, 1024),
        'w5T4': (256, 1024), 'b5c': (128, 8), 'b5c2': (128, 8),
        'l1wT': (2048, 512), 'b6c': (128, 4), 'b6c2': (128, 4),
        'l2wT': (512, 256), 'b2c': (128, 2), 'b2c2': (128, 2),
        'l3wT': (256, 40), 'b3c': (128, 1),
        'id128': (128, 128),
        'cones': (128, 1), 'conesrow': (1, 128),
    })
    return specs


# --------------------------------------------------------------- build kernel
class G:
    pass


def load_one_weight(g, pool, name, shape):
    nc = g.nc
    kind, off, _ = g.blob_lay[name]
    dt = BF16 if name in BF16_W else (F32R if (RDT and name in F32R_W) else F32)
    n = int(np.prod(shape))
    src = g.win['wblob16' if kind == 'bf16' else 'wblob32'].ap()[off:off + n]
    if len(shape) == 2 and shape[0] > 128:
        kc = shape[0] // 128
        t = pool.tile([128, kc, shape[1]], dt, name=f'w_{name}')
        nc.sync.dma_start(
            out=t[:], in_=src.rearrange('(k p n) -> p k n', p=128, n=shape[1]))
        g.wkc[name] = kc
    else:
        t = pool.tile(list(shape), dt, name=f'w_{name}')
        nc.sync.dma_start(
            out=t[:], in_=src.rearrange('(p n) -> p n', p=shape[0]))
        g.wkc[name] = 0
    g.w[name] = t


def load_weights_sbuf(g):
    g.w = {}
    g.wkc = {}          # number of 128-row chunks for >128-row 2D weights
    for name, shape in weight_specs().items():
        if name in HEAD_W:
            continue
        load_one_weight(g, g.wpool, name, shape)


def wsl(g, name, r0, p, c0=None, cn=None):
    """Slice rows [r0:r0+p] (and cols [c0:c0+cn]) of a stored weight."""
    t = g.w[name]
    if g.wkc[name]:
        assert r0 % 128 + p <= 128
        ap = t[r0 % 128:r0 % 128 + p, r0 // 128, :]
    else:
        ap = t[r0:r0 + p, :]
    if c0 is not None:
        ap = ap[:, c0:c0 + cn]
    return ap


def dbg_emit(g, name, ap):
    if g.debug != name:
        return
    g.dbg_out = g.nc.dram_tensor('dbg', list(ap.shape), ap.dtype, kind='ExternalOutput')
    g.nc.sync.dma_start(out=g.dbg_out.ap(), in_=ap)
    raise StopBuild()


def stt2(g, out, in0, in1, op):
    """out = in0 <op> in1 on DVE (TensorTensor: 2x for packed 16-bit)."""
    g.nc.vector.tensor_tensor(out=out, in0=in0, in1=in1, op=op)


def tree_reduce(g, slicer, n, op):
    """In-place pairwise reduction along one axis; result lands at index 0.

    slicer(lo, hi) must return the AP slice [lo:hi) of the tree axis.
    """
    while n > 1:
        h = n // 2
        stt2(g, slicer(0, h), slicer(0, h), slicer(h, 2 * h), op)
        if n % 2:
            stt2(g, slicer(0, 1), slicer(0, 1), slicer(n - 1, n), op)
        n = h


def mm_acc(g, ps_ap, pairs, nmax=512, rdt=RDT):
    """Accumulate sum_i lhsT_i.T @ rhs_i into psum AP, splitting free dim.

    rdt=True runs the PE in float32r (fast fp32) mode.
    """
    nc = g.nc
    Nfree = pairs[0][1].shape[-1]
    for c0 in range(0, Nfree, nmax):
        cw = min(nmax, Nfree - c0)
        for j, (lt, rh) in enumerate(pairs):
            if rdt and lt.dtype == F32 and rh.dtype == F32:
                lt = lt.bitcast(F32R)
                rh = rh.bitcast(F32R)
            nc.tensor.matmul(
                out=ps_ap[:, c0:c0 + cw], lhsT=lt, rhs=rh[:, c0:c0 + cw],
                start=(j == 0), stop=(j == len(pairs) - 1))


def edgeconv(g, li, out_slices):
    """Token-sharded EdgeConv.

    Reads g.XF (C, 8, 1024) full activations and g.XL (list of local
    feature-major slices, total C rows of (rows, 8, 128)). Writes
    lrelu(max_k Wn x_nbr + Wz x_loc + b) into out_slices ((rows, 8, 128)).
    """
    nc, tc = g.nc, g.tc
    C, O = EC[li - 1]
    nO = (O + 127) // 128
    XF = g.XF                                          # (C, 8, 1024) AP

    with tc.tile_pool(name=f'ec{li}', bufs=1) as pool, \
         tc.tile_pool(name=f'ec{li}_ps', bufs=1, space='PSUM') as psp:

        # ---- 2*x_loc (lhsT for s)
        lhs2x = pool.tile([C, 8, TOK], BF16, name=f'l2x{li}')
        r = 0
        for xs in g.XL:
            p = xs.shape[0]
            nc.scalar.activation(out=lhs2x[r:r + p, :, :], in_=xs,
                                 func=ACTF.Copy, scale=2.0)
            r += p

        # ---- z = Wz @ x_loc (local tokens only)
        z_sb = pool.tile([128, nO, 8, TOK], F32, name=f'z_{li}')
        for oc in range(nO):
            ow = min(128, O - oc * 128)
            ps = psp.tile([128, 8 * TOK], F32, name=f'zps{li}', tag='sps', bufs=2)
            mm_acc(g, ps[0:ow, :],
                   [(wsl(g, f'ec{li}_wzT', 0, C, oc * 128, ow),
                     lhs2x.rearrange('c b n -> c (b n)'))])
            # lhs2x holds 2*x_loc; halve via scale on the copy out
            nc.scalar.activation(
                out=z_sb[0:ow, oc, :, :].rearrange('p b n -> p (b n)'),
                in_=ps[0:ow, :], func=ACTF.Copy, scale=0.5)

        # ---- per-sample pipeline: s -> packed top-k -> idx wrap -> y ->
        #      gather -> neighbor-max (wrap is per-sample so the gather of
        #      sample b overlaps the top-k of sample b+1)
        I24 = pool.tile([128, 8, 24], U16, name=f'i24_{li}')
        IDX = pool.tile([128, 8, 8 * KNN], I16, name=f'idx_{li}')
        m_sb = pool.tile([128, nO, 8, TOK], F32, name=f'm_{li}')
        for b in range(8):
            x2 = pool.tile([C, N], BF16, name=f'x2_{li}', tag='x2', bufs=2)
            nc.scalar.activation(out=x2[:], in_=XF[:, b, :], func=ACTF.Square)
            s_ps = psp.tile([128, N], F32, name=f's{li}', tag='sps', bufs=2)
            mm_acc(g, s_ps[:], [(lhs2x[:, b, :], XF[:, b, :]),
                                (g.negC16[0:C, :], x2[:])])
            if g.debug == f's{li}' and b == 0:
                dbg_emit(g, f's{li}', s_ps[:])
            SP = pool.tile([128, N], F32, name=f'sp_{li}', tag='sp', bufs=2)
            nc.vector.scalar_tensor_tensor(
                out=SP[:].bitcast(I32), in0=s_ps[:].bitcast(I32),
                scalar=g.maskhi[:], in1=g.iota[:],
                op0=ALU.bitwise_and, op1=ALU.bitwise_or)
            cand = pool.tile([128, 64], F32, name=f'cd_{li}', tag='cand', bufs=2)
            for seg in range(8):
                nc.vector.max(out=cand[:, seg * 8:seg * 8 + 8],
                              in_=SP[:, seg * 128:(seg + 1) * 128])
            P24 = pool.tile([128, 24], F32, name=f'p24_{li}', tag='p24', bufs=2)
            nc.vector.max(out=P24[:, 0:8], in_=cand[:])
            nc.vector.match_replace(out=cand[:], in_to_replace=P24[:, 0:8],
                                    in_values=cand[:], imm_value=-1e30)
            nc.vector.max(out=P24[:, 8:16], in_=cand[:])
            nc.vector.match_replace(out=cand[:], in_to_replace=P24[:, 8:16],
                                    in_values=cand[:], imm_value=-1e30)
            nc.vector.max(out=P24[:, 16:24], in_=cand[:])
            nc.vector.tensor_scalar(
                out=I24[:, b, :],
                in0=P24[:].bitcast(U16).rearrange(
                    'p (k two) -> p k two', two=2)[:, :, 0],
                scalar1=1023, scalar2=None, op0=ALU.bitwise_and)

            # wrap: IDX[16G+q, b, g*20+k] = idx[g*16+q, b, k].
            # DMA1 writes DRAM already in wrapped [q, gg, k] order; DMA2
            # reads the whole block 8x (stride-0 lead dim) to fill all 8
            # 16-partition replica groups in one transfer.
            idx_dram = g.dram.tile([16, 8, KNN], I16, name=f'idxd{li}_{b}')
            nc.sync.dma_start(
                out=idx_dram[:].rearrange('q gg k -> gg q k'),
                in_=I24[:, b, 0:KNN].bitcast(I16))
            nc.sync.dma_start(
                out=IDX[:, b, :],
                in_=idx_dram[:].rearrange('(o q) gg k -> o q gg k', o=1)
                .to_broadcast([8, 16, 8, KNN]))

            # y for this sample, gather, neighbor-max
            if li < 4:
                y_sb = pool.tile([128, N], F32, name=f'y_{li}', tag='ysb',
                                 bufs=2)
                ps = psp.tile([128, N], F32, name=f'yps{li}', tag='yps', bufs=2)
                mm_acc(g, ps[0:O, :],
                       [(wsl(g, f'ec{li}_wnT', 0, C, 0, O), XF[:, b, :])])
                nc.scalar.activation(out=y_sb[0:O, :], in_=ps[0:O, :],
                                     func=ACTF.Copy)
                gth = pool.tile([128, TOK * KNN], F32, name=f'g_{li}',
                                tag='gth', bufs=2)
                nc.gpsimd.ap_gather(
                    out_ap=gth[0:O, :].rearrange('p (f o) -> p f o', o=1),
                    in_ap=y_sb[0:O, :].rearrange('p (f o) -> p f o', o=1),
                    idxs_ap=IDX[0:O, b, :],
                    channels=O, num_elems=N, d=1, num_idxs=TOK * KNN)
                nc.vector.tensor_reduce(
                    out=m_sb[0:O, 0, b, :].rearrange('p (gg q) -> p gg q',
                                                     gg=8),
                    in_=gth[0:O, :].rearrange('p (gg k q) -> p gg q k',
                                              gg=8, k=KNN, q=16),
                    axis=AX.X, op=ALU.max)
            else:
                # layer 4: two oc planes interleaved, bf16, one d=2 gather
                y_sb = pool.tile([128, N, 2], BF16, name=f'y_{li}',
                                 tag='ysb', bufs=2)
                for oc in range(2):
                    ps = psp.tile([128, N], F32, name=f'yps{li}', tag='yps',
                                  bufs=2)
                    mm_acc(g, ps[:],
                           [(wsl(g, f'ec{li}_wnT', 0, C, oc * 128, 128),
                             XF[:, b, :])])
                    nc.scalar.activation(out=y_sb[:, :, oc], in_=ps[:],
                                         func=ACTF.Copy)
                gth = pool.tile([128, TOK * KNN, 2], BF16, name=f'g_{li}',
                                tag='gth', bufs=2)
                nc.gpsimd.ap_gather(
                    out_ap=gth[:], in_ap=y_sb[:],
                    idxs_ap=IDX[:, b, :],
                    channels=128, num_elems=N, d=2, num_idxs=TOK * KNN)
                gv = gth[:].rearrange('p (gg k q) two -> p gg k q two',
                                      gg=8, k=KNN, q=16)
                tree_reduce(g, lambda lo, hi: gv[:, :, lo:hi, :, :],
                            KNN, ALU.max)
                stt2(g, m_sb[:, :, b, :].rearrange(
                         'p two (gg q) -> p gg q two', gg=8),
                     gv[:, :, 0, :, :], gv[:, :, 0, :, :], ALU.max)
        dbg_emit(g, f'i24_{li}', I24[:])
        dbg_emit(g, f'idx_{li}', IDX[:])
        dbg_emit(g, f'm_{li}', m_sb[0:min(O, 128), :, :, :])

        # ---- epilogue: lrelu(m + z + b) -> out slices, per sample-half so
        #      the next stage's qkv can start while late gathers still run
        for hb in range(2):
            bsl = slice(hb * 4, hb * 4 + 4)
            for oc in range(nO):
                ow = min(128, O - oc * 128)
                t_sb = pool.tile([128, 4, TOK], F32, name=f't_{li}',
                                 tag='tsb', bufs=2)
                stt2(g, t_sb[0:ow, :, :], m_sb[0:ow, oc, bsl, :],
                     z_sb[0:ow, oc, bsl, :], ALU.add)
                neg = pool.tile([128, 4, TOK], BF16, name=f'neg_{li}',
                                tag='negt', bufs=2)
                r = 0
                for os_ in out_slices:
                    p = os_.shape[0]
                    lo, hi = max(r, oc * 128), min(r + p, oc * 128 + ow)
                    if lo < hi:
                        sl = slice(lo - oc * 128, hi - oc * 128)
                        nc.scalar.activation(
                            out=os_[lo - r:hi - r, bsl, :], in_=t_sb[sl, :, :],
                            func=ACTF.Relu,
                            bias=g.w[f'ec{li}_b'][sl, oc:oc + 1])
                        nc.scalar.activation(
                            out=neg[sl, :, :], in_=t_sb[sl, :, :],
                            func=ACTF.Relu, scale=-0.2,
                            bias=g.w[f'ec{li}_b2'][sl, oc:oc + 1])
                        stt2(g, os_[lo - r:hi - r, bsl, :],
                             os_[lo - r:hi - r, bsl, :],
                             neg[sl, :, :], ALU.subtract)
                    r += p


def mha(g, li, x_slices, on_half=None):
    """Local cross-batch MHA + residual, in place on x_slices ((rows, 8, 128)).

    Token-major: qkv (128tok, 8sample, 3E). Attention mixes over the sample
    axis m (all resident). After qkv, the q-side work runs in two sample
    halves; on_half(h) fires when a half's residual is in place so its
    AllGather can overlap the other half's compute.
    """
    nc, tc = g.nc, g.tc
    E = ES[li - 1]
    d = E // H
    ne = (E + 127) // 128

    with tc.tile_pool(name=f'mha{li}', bufs=1) as pool, \
         tc.tile_pool(name=f'mha{li}_ps', bufs=1, space='PSUM') as psp:

        # ---- qkv[p, b, o] = sum_e x[e, b, p] wiT[e, o] + bi[o]  (bf16)
        qkv = pool.tile([128, 8, 3 * E], BF16, name=f'qkv_{li}')
        for b in range(8):
            ps = psp.tile([128, 3 * E], F32, name=f'qk{li}', tag='qkvps', bufs=2)
            pairs = []
            r = 0
            for xs in x_slices:
                p = xs.shape[0]
                pairs.append((xs[:, b, :], wsl(g, f'a{li}_wiT', r, p)))
                r += p
            pairs.append((g.onesrow[:], g.w[f'a{li}_biR'][:]))
            mm_acc(g, ps[:], pairs)
            nc.scalar.activation(out=qkv[:, b, :], in_=ps[:], func=ACTF.Copy)
        dbg_emit(g, f'qkv_{li}', qkv[:])

        q = qkv[:, :, 0:E]
        k = qkv[:, :, E:2 * E]
        v = qkv[:, :, 2 * E:3 * E]
        o_fm = pool.tile([128, ne, 8, TOK], BF16, name=f'of_{li}')

        for half in range(2):
            b0 = half * 4
            bsl = slice(b0, b0 + 4)

            # scores: prod = q*k (bf16 2x), then pairwise d-tree sum
            prod = pool.tile([128, 4, 8, E], BF16, name=f'pr_{li}',
                             tag='prod', bufs=2)
            stt2(g, prod[:],
                 q[:, bsl, :].rearrange('p b (o e) -> p b o e', o=1)
                 .to_broadcast([128, 4, 8, E]),
                 k.rearrange('p m (o e) -> p o m e', o=1).to_broadcast(
                     [128, 4, 8, E]),
                 ALU.mult)
            pv = prod[:].rearrange('p b m (h dd) -> p b m h dd', h=H)
            tree_reduce(g, lambda lo, hi: pv[:, :, :, :, lo:hi], d, ALU.add)
            st = pv[:, :, :, :, 0]                   # (128, 4, 8, H) strided

            # softmax over m (denominator folded into the output scale)
            ex = pool.tile([128, 4, 8, H], BF16, name=f'ex_{li}',
                           tag='ex', bufs=2)
            nc.scalar.activation(out=ex[:], in_=st, func=ACTF.Exp)
            sm = pool.tile([128, 4, H], F32, name=f'sm_{li}', tag='sm', bufs=2)
            nc.vector.tensor_reduce(
                out=sm[:], in_=ex[:].rearrange('p b m h -> p b h m'),
                axis=AX.X, op=ALU.add)
            rec = pool.tile([128, 4, H], F32, name=f'rc_{li}', tag='rc', bufs=2)
            nc.vector.reciprocal(out=rec[:], in_=sm[:])

            # o[p, b, e] = (sum_m ex[b,m,h(e)] v[m,e]) * rec[b,h(e)]
            prod2 = pool.tile([128, 4, 8, E], BF16, name=f'p2_{li}',
                              tag='prod', bufs=2)
            stt2(g, prod2[:].rearrange('p b m (h dd) -> p b m h dd', h=H),
                 v.rearrange('p m (o h dd) -> p o m h dd', o=1, h=H)
                 .to_broadcast([128, 4, 8, H, d]),
                 ex[:].rearrange('p b m (h o) -> p b m h o', o=1).to_broadcast(
                     [128, 4, 8, H, d]),
                 ALU.mult)
            tree_reduce(g, lambda lo, hi: prod2[:, :, lo:hi, :], 8, ALU.add)
            o_t = pool.tile([128, 4, E], BF16, name=f'o_{li}', tag='ot', bufs=2)
            stt2(g, o_t[:].rearrange('p b (h dd) -> p b h dd', h=H),
                 prod2[:, :, 0, :].rearrange('p b (h dd) -> p b h dd', h=H),
                 rec[:].rearrange('p b (h o) -> p b h o', o=1).to_broadcast(
                     [128, 4, H, d]),
                 ALU.mult)

            # transpose o to feature-major, project, bias, residual
            for ec in range(ne):
                p = min(128, E - ec * 128)
                po = psp.tile([128, 4 * TOK], BF16, name=f'po{li}',
                              tag='pops', bufs=1)
                for j in range(4):
                    nc.tensor.transpose(out=po[0:p, j * TOK:(j + 1) * TOK],
                                        in_=o_t[:, j, ec * 128:ec * 128 + p],
                                        identity=g.w['id128'][:])
                nc.scalar.activation(
                    out=o_fm[0:p, ec, bsl, :].rearrange('p b n -> p (b n)'),
                    in_=po[0:p, :], func=ACTF.Copy)
            for oc in range(ne):
                p = min(128, E - oc * 128)
                ps = psp.tile([128, 4 * TOK], F32, name=f'pj{li}',
                              tag='pjps', bufs=2)
                pairs = []
                for ec in range(ne):
                    pk = min(128, E - ec * 128)
                    pairs.append(
                        (wsl(g, f'a{li}_woT', ec * 128, pk, oc * 128, p),
                         o_fm[0:pk, ec, bsl, :].rearrange('p b n -> p (b n)')))
                mm_acc(g, ps[0:p, :], pairs)
                tmp = pool.tile([128, 4, TOK], BF16, name=f'pt_{li}',
                                tag='ptmp', bufs=2)
                nc.scalar.activation(
                    out=tmp[0:p, :, :].rearrange('p b n -> p (b n)'),
                    in_=ps[0:p, :], func=ACTF.Identity,
                    bias=g.w[f'a{li}_boc'][0:p, oc:oc + 1])
                r = 0
                for xs in x_slices:
                    pp = xs.shape[0]
                    lo, hi = max(r, oc * 128), min(r + pp, oc * 128 + p)
                    if lo < hi:
                        stt2(g, xs[lo - r:hi - r, bsl, :],
                             tmp[lo - oc * 128:hi - oc * 128, :, :],
                             xs[lo - r:hi - r, bsl, :], ALU.add)
                    r += pp
            if on_half is not None:
                on_half(half)


def ag_half(g, li, out_slices, XFc, half):
    """AllGather one 4-sample half of the bf16 (O, 8, 128) local shard into
    XFc (O, 8, 1024). Fired from mha's on_half so the first half's
    collective overlaps the second half's attention compute, and the second
    overlaps the next edgeconv's leading samples."""
    nc = g.nc
    xs = out_slices[0]
    rows = xs.shape[0]
    b0 = half * 4
    xc = g.dram.tile([rows, 4, TOK], BF16, name=f'xc{li}_{half}')
    nc.sync.dma_start(out=xc[:], in_=xs[:, b0:b0 + 4, :])
    x_all = g.dram.tile([NCORES, rows, 4, TOK], BF16,
                        name=f'xall{li}_{half}', addr_space='Shared')
    nc.gpsimd.collective_compute(
        'AllGather', ALU.bypass, replica_groups=RG,
        ins=[xc[:].opt()], outs=[x_all[:].opt()])
    nc.sync.dma_start(
        out=XFc[0:rows, b0:b0 + 4, :].rearrange(
            'o b (c n) -> o b c n', c=NCORES),
        in_=x_all[:].rearrange('c o b n -> o b c n'))


def head(g):
    nc, tc = g.nc, g.tc
    specs = weight_specs()
    with tc.tile_pool(name='head', bufs=1) as pool, \
         tc.tile_pool(name='head_ps', bufs=1, space='PSUM') as psp:
        for name in HEAD_W:
            load_one_weight(g, pool, name, specs[name])
        cat = g.cat
        h_sb = pool.tile([128, 8, 8, TOK], F32, name='h5')
        hneg = pool.tile([128, 8, TOK], F32, name='h5n', tag='h5n', bufs=2)
        for oc in range(8):
            ps = psp.tile([128, 8 * TOK], F32, name='h5_ps', tag='h5ps', bufs=2)
            cs = slice(oc * 128, oc * 128 + 128)
            pairs = [(g.w['w5T1'][:, cs], cat[0:64, 0, :, :].rearrange(
                        'p b n -> p (b n)')),
                     (g.w['w5T2'][:, cs], cat[0:64, 1, :, :].rearrange(
                        'p b n -> p (b n)')),
                     (g.w['w5T3'][:, cs], cat[0:128, 2, :, :].rearrange(
                        'p b n -> p (b n)')),
                     (g.w['w5T4'][:, 0, cs], cat[0:128, 3, :, :].rearrange(
                        'p b n -> p (b n)')),
                     (g.w['w5T4'][:, 1, cs], cat[0:128, 4, :, :].rearrange(
                        'p b n -> p (b n)'))]
            mm_acc(g, ps[:], pairs)
            nc.scalar.activation(
                out=h_sb[:, oc, :, :].rearrange('p b n -> p (b n)'), in_=ps[:],
                func=ACTF.Relu, bias=g.w['b5c'][:, oc:oc + 1])
            nc.scalar.activation(
                out=hneg[:].rearrange('p b n -> p (b n)'), in_=ps[:],
                func=ACTF.Relu, scale=-0.2, bias=g.w['b5c2'][:, oc:oc + 1])
            stt2(g, h_sb[:, oc, :, :], h_sb[:, oc, :, :], hneg[:], ALU.subtract)
        # per-sample local max / sum partials over the 128 local tokens
        hp = pool.tile([128, 8, 8, 2], F32, name='hpart')
        for oc in range(8):
            nc.vector.tensor_reduce(out=hp[:, oc, :, 0],
                                    in_=h_sb[:, oc, :, :], axis=AX.X, op=ALU.max)
            nc.vector.tensor_reduce(out=hp[:, oc, :, 1],
                                    in_=h_sb[:, oc, :, :], axis=AX.X, op=ALU.add)
        dbg_emit(g, 'hpart', hp[:])
        hc = g.dram.tile([128, 8, 8, 2], F32, name='hc')
        nc.sync.dma_start(out=hc[:], in_=hp[:])
        h_all = g.dram.tile([NCORES, 128, 8, 8, 2], F32, name='hall',
                            addr_space='Shared')
        nc.gpsimd.collective_compute(
            'AllGather', ALU.bypass, replica_groups=RG,
            ins=[hc[:].opt()], outs=[h_all[:].opt()])
        hl = pool.tile([128, NCORES, 8, 8, 2], F32, name='hload')
        nc.sync.dma_start(out=hl[:],
                          in_=h_all[:].rearrange('c p o b s -> p c o b s'))
        hmax = pool.tile([128, 8, 8], F32, name='hmax')
        hsum = pool.tile([128, 8, 8], F32, name='hsum')
        nc.vector.tensor_reduce(
            out=hmax[:], in_=hl[:, :, :, :, 0].rearrange('p c o b -> p o b c'),
            axis=AX.X, op=ALU.max)
        nc.vector.tensor_reduce(
            out=hsum[:], in_=hl[:, :, :, :, 1].rearrange('p c o b -> p o b c'),
            axis=AX.X, op=ALU.add)
        dbg_emit(g, 'hmax', hmax[:])
        dbg_emit(g, 'hsum', hsum[:])

        fc1 = pool.tile([128, 4, 8], F32, name='fc1')
        for oc in range(4):
            ps = psp.tile([128, 8], F32, name='fc_ps', tag='fcps', bufs=2)
            for kc in range(16):
                rhs = hmax[:, kc, :] if kc < 8 else hsum[:, kc - 8, :]
                nc.tensor.matmul(out=ps[:],
                                 lhsT=g.w['l1wT'][:, kc, oc * 128:oc * 128 + 128],
                                 rhs=rhs, start=(kc == 0), stop=(kc == 15))
            fneg = pool.tile([128, 8], F32, name='fc1n', tag='fcn', bufs=2)
            nc.scalar.activation(out=fc1[:, oc, :], in_=ps[:], func=ACTF.Relu,
                                 bias=g.w['b6c'][:, oc:oc + 1])
            nc.scalar.activation(out=fneg[:], in_=ps[:], func=ACTF.Relu, scale=-0.2,
                                 bias=g.w['b6c2'][:, oc:oc + 1])
            nc.vector.tensor_tensor(out=fc1[:, oc, :], in0=fc1[:, oc, :],
                                    in1=fneg[:], op=ALU.subtract)
        fc2 = pool.tile([128, 2, 8], F32, name='fc2')
        for oc in range(2):
            ps = psp.tile([128, 8], F32, name='fc_ps2', tag='fcps', bufs=2)
            for kc in range(4):
                nc.tensor.matmul(out=ps[:],
                                 lhsT=g.w['l2wT'][:, kc, oc * 128:oc * 128 + 128],
                                 rhs=fc1[:, kc, :], start=(kc == 0), stop=(kc == 3))
            fneg = pool.tile([128, 8], F32, name='fc2n', tag='fcn', bufs=2)
            nc.scalar.activation(out=fc2[:, oc, :], in_=ps[:], func=ACTF.Relu,
                                 bias=g.w['b2c'][:, oc:oc + 1])
            nc.scalar.activation(out=fneg[:], in_=ps[:], func=ACTF.Relu, scale=-0.2,
                                 bias=g.w['b2c2'][:, oc:oc + 1])
            nc.vector.tensor_tensor(out=fc2[:, oc, :], in0=fc2[:, oc, :],
                                    in1=fneg[:], op=ALU.subtract)
        ps = psp.tile([128, 8], F32, name='fc_ps3', tag='fcps', bufs=2)
        for kc in range(2):
            nc.tensor.matmul(out=ps[0:40, :], lhsT=g.w['l3wT'][:, kc, 0:40],
                             rhs=fc2[:, kc, :], start=(kc == 0), stop=(kc == 1))
        outs = pool.tile([128, 8], F32, name='outs')
        nc.scalar.activation(out=outs[0:40, :], in_=ps[0:40, :], func=ACTF.Identity,
                             bias=g.w['b3c'][0:40, :])
        nc.sync.dma_start(out=g.out.ap(), in_=outs[0:40, :])


def build(debug=None):
    g = G()
    g.debug = debug
    nc = bacc.Bacc('TRN2', target_bir_lowering=False, debug=False,
                   num_devices=NCORES)
    g.nc = nc
    g.win = {}
    g.win['xfull'] = nc.dram_tensor('xfull', [3, 8, N], BF16, kind='ExternalInput')
    g.win['xloc'] = nc.dram_tensor('xloc', [3, 8, TOK], BF16, kind='ExternalInput')
    g.blob_lay, blob_sz = blob_layout()
    g.win['wblob16'] = nc.dram_tensor('wblob16', [blob_sz['bf16']], BF16,
                                      kind='ExternalInput')
    g.win['wblob32'] = nc.dram_tensor('wblob32', [blob_sz['f32']], F32,
                                      kind='ExternalInput')
    if debug is None:
        g.out = nc.dram_tensor('out', [40, 8], F32, kind='ExternalOutput')

    with TileContext(nc) as tc:
        g.tc = tc
        with tc.tile_pool(name='wpool', bufs=1) as wpool, \
             tc.tile_pool(name='gpool', bufs=1) as gpool, \
             tc.tile_pool(name='dram', bufs=1, space='DRAM') as dram:
            g.wpool, g.dram = wpool, dram
            try:
                load_weights_sbuf(g)
                g.ones = g.w['cones']
                g.negC16 = gpool.tile([128, 128], BF16, name='negC16')
                nc.vector.memset(g.negC16[:], -1.0)
                g.onesrow = g.w['conesrow']
                g.iota = gpool.tile([128, N], I32, name='iota')
                nc.gpsimd.iota(g.iota[:], pattern=[[1, N]], channel_multiplier=0)
                g.maskhi = gpool.tile([128, 1], I32, name='maskhi')
                nc.vector.memset(g.maskhi[:], -1024)       # 0xFFFFFC00
                g.masklo = gpool.tile([128, 1], I32, name='masklo')
                nc.vector.memset(g.masklo[:], 1023)
                g.cat = gpool.tile([128, 5, 8, TOK], BF16, name='cat')
                if debug is not None:
                    nc.vector.memset(g.cat[:], 0.0)

                x1 = [g.cat[0:64, 0, :, :]]
                x2 = [g.cat[0:64, 1, :, :]]
                x3 = [g.cat[0:128, 2, :, :]]
                x4 = [g.cat[0:128, 3, :, :], g.cat[0:128, 4, :, :]]

                xfctx = tc.tile_pool(name='xfpool', bufs=1)
                xfpool = xfctx.__enter__()
                XF2c = xfpool.tile([64, 8, N], BF16, name='XF2c')
                XF3c = xfpool.tile([64, 8, N], BF16, name='XF3c')
                XF4c = xfpool.tile([128, 8, N], BF16, name='XF4c')

                with tc.tile_pool(name='l1', bufs=1) as p1:
                    XF1 = p1.tile([3, 8, N], BF16, name='XF1')
                    nc.sync.dma_start(out=XF1[:], in_=g.win['xfull'].ap())
                    XL1 = p1.tile([3, 8, TOK], BF16, name='XL1')
                    nc.sync.dma_start(out=XL1[:], in_=g.win['xloc'].ap())
                    g.XF, g.XL = XF1[:], [XL1[:]]
                    edgeconv(g, 1, x1)
                    dbg_emit(g, 'x1', g.cat[:])
                    mha(g, 1, x1,
                        on_half=lambda h: ag_half(g, 1, x1, XF2c, h))
                    dbg_emit(g, 'xa1', g.cat[:])

                g.XF, g.XL = XF2c[0:64, :, :], x1
                edgeconv(g, 2, x2)
                dbg_emit(g, 'x2', g.cat[:])
                mha(g, 2, x2, on_half=lambda h: ag_half(g, 2, x2, XF3c, h))
                dbg_emit(g, 'xa2', g.cat[:])

                g.XF, g.XL = XF3c[0:64, :, :], x2
                edgeconv(g, 3, x3)
                dbg_emit(g, 'x3', g.cat[:])
                mha(g, 3, x3, on_half=lambda h: ag_half(g, 3, x3, XF4c, h))
                dbg_emit(g, 'xa3', g.cat[:])

                g.XF, g.XL = XF4c[0:128, :, :], x3
                edgeconv(g, 4, x4)
                dbg_emit(g, 'x4', g.cat[:])
                mha(g, 4, x4)
                dbg_emit(g, 'cat', g.cat[:])
                xfctx.__exit__(None, None, None)
                head(g)
            except StopBuild:
                pass
    nc.compile()
    return g


# ------------------------------------------------------------------ host run
def make_in_maps(inputs):
    w = prep_weights(inputs)
    lay, sz = blob_layout()
    blob = {'bf16': np.zeros((sz['bf16'],), np.dtype('bfloat16')),
            'f32': np.zeros((sz['f32'],), np.float32)}
    for name, (kind, off, shape) in lay.items():
        a = np.asarray(w[name]).ravel()
        blob[kind][off:off + a.size] = a
    x = np.asarray(inputs['x'], np.float32)           # (8, 1024, 3)
    xf = np.ascontiguousarray(x.transpose(2, 0, 1)).astype(
        np.dtype('bfloat16'))                         # (3, 8, 1024)
    in_maps = []
    for c in range(NCORES):
        m = {'xfull': xf,
             'xloc': np.ascontiguousarray(xf[:, :, c * TOK:(c + 1) * TOK]),
             'wblob16': blob['bf16'], 'wblob32': blob['f32']}
        in_maps.append(m)
    return in_maps


def kernel(**inputs):
    g = build()
    in_maps = make_in_maps(inputs)
    res = bass_utils.run_bass_kernel_spmd(g.nc, in_maps, core_ids=list(range(NCORES)))
    return np.ascontiguousarray(
        np.asarray(res.results[0]['out']).reshape(40, 8).T).astype(np.float32)

